# revision 27
# baseline (speedup 1.0000x reference)
"""Two-layer modulated deformable conv (DCNv2) + sync-BN + ReLU for trn2.

The two big contractions (~39 GFLOP each) run in bf16 (fp32 PSUM) on 8
NeuronCores, data-parallel over (batch, H-half) -> 8 shards; offsets/BN and
the data-dependent bilinear gathers stay on the host.

Layer 1 exploits that bilinear sampling is linear in x and commutes with the
channel contraction: the device computes per-tap U_k = W_k @ x from the
UNEXPANDED input (24.5 MB/core of HBM traffic instead of 40 MB for the
9x-expanded im2col), and the host samples/modulates U. Layer 2 has the
opposite shape asymmetry (C=128 < O=256), so the host expands the modulated
samples and the device does the [256,1152]@[1152,8192] GEMM.

Both Bass kernels stream pre-packed partition-contiguous slabs (HWDGE,
per-buffer-slot semaphores since a DMA's +16 is 16 per-engine increments),
keep PE back-to-back via 8-bank PSUM rings, and split PSUM eviction across
the vector and scalar engines where eviction would otherwise bind.
"""

import numpy as np

B, CIN, H, W = 4, 256, 128, 128
MID, COUT = 128, 256
HW = H * W
K2 = 9
_EPS = 1e-5

_KY = np.array([-1, -1, -1, 0, 0, 0, 1, 1, 1], dtype=np.float32)
_KX = np.array([-1, 0, 1, -1, 0, 1, -1, 0, 1], dtype=np.float32)


# ---------------------------------------------------------------- host pieces
def _im2col(x):
    """x [B,C,H,W] -> cols [B, C*9, H*W] (3x3 SAME, zero pad)."""
    b, c, h, w = x.shape
    xp = np.zeros((b, c, h + 2, w + 2), dtype=x.dtype)
    xp[:, :, 1:-1, 1:-1] = x
    cols = np.empty((b, c, 9, h, w), dtype=x.dtype)
    k = 0
    for dy in range(3):
        for dx in range(3):
            cols[:, :, k] = xp[:, :, dy:dy + h, dx:dx + w]
            k += 1
    return cols.reshape(b, c * 9, h * w)


def _conv3x3_host(cols, w, bias):
    """cols [B, C*9, HW], w [O,C,3,3] -> [B, O, HW]."""
    o = w.shape[0]
    wr = w.reshape(o, -1)
    out = np.matmul(wr[None], cols)  # [B, O, HW]
    return out + bias[None, :, None]


def _bilinear_modulated(x, py, px, mask):
    """x [C,H,W]; py,px,mask [9,H,W] -> modulated samples [C*9, HW]."""
    c, h, w = x.shape
    y0 = np.floor(py)
    x0 = np.floor(px)
    ly = py - y0
    lx = px - x0
    y0i = y0.astype(np.int32)
    x0i = x0.astype(np.int32)
    flat = x.reshape(c, h * w)

    def gather(yi, xi):
        valid = ((yi >= 0) & (yi < h) & (xi >= 0) & (xi < w)).astype(np.float32)
        idx = np.clip(yi, 0, h - 1) * w + np.clip(xi, 0, w - 1)
        v = flat[:, idx.reshape(-1)].reshape(c, *yi.shape)
        return v * valid[None]

    v00 = gather(y0i, x0i)
    v01 = gather(y0i, x0i + 1)
    v10 = gather(y0i + 1, x0i)
    v11 = gather(y0i + 1, x0i + 1)
    w00 = ((1 - ly) * (1 - lx) * mask)[None]
    w01 = ((1 - ly) * lx * mask)[None]
    w10 = (ly * (1 - lx) * mask)[None]
    w11 = (ly * lx * mask)[None]
    s = v00 * w00 + v01 * w01 + v10 * w10 + v11 * w11  # [C,9,H,W]
    return s.reshape(c * 9, h * w).astype(np.float32)


def _sampled_for_layer(x, w_off, b_off):
    """x [B,C,H,W] -> modulated sampled cols [B, C*9, HW]."""
    b, c, h, w = x.shape
    om = _conv3x3_host(_im2col(x), w_off, b_off).reshape(b, 27, h, w)
    off_y = om[:, :K2]
    off_x = om[:, K2:2 * K2]
    mask = 1.0 / (1.0 + np.exp(-om[:, 2 * K2:]))
    yy = np.arange(h, dtype=np.float32)
    xx = np.arange(w, dtype=np.float32)
    py = yy[None, None, :, None] + _KY[None, :, None, None] + off_y  # [B,9,H,W]
    px = xx[None, None, None, :] + _KX[None, :, None, None] + off_x
    out = np.empty((b, c * 9, h * w), dtype=np.float32)
    for i in range(b):
        out[i] = _bilinear_modulated(x[i], py[i], px[i], mask[i])
    return out


# --------------------------------------------------- layer 1: sample-after-GEMM
# Bilinear sampling is linear in x, so it commutes with the channel
# contraction: y1[o,p] = sum_k m_k(p) * bilin(U_k[o,:], p+d_k(p)) with
# U_k = W_k @ x. The device computes U (no 9x im2col blow-up on the DMA-in
# side, and 9*O*N out < 9*C*N in since O=128 < C=256); the host does the
# cheap data-dependent sampling on U.
_HALO = 2
_HS = 64 + 2 * _HALO          # shard rows incl halo
_NW = _HS * W                 # device ncols per shard (17 * 512)


def _device_taps_l1(x, wall):
    """x [B,CIN,H,W], wall [9*MID, CIN] (row k*MID+o = w1[o,:,k]).
    Returns per-shard U [9*MID, _NW] float32, shard s = (b=s//2, hh=s%2)."""
    import time

    import ml_dtypes
    from concourse import bass_utils

    bf16 = np.dtype(ml_dtypes.bfloat16)
    kdim, odim, ncols = CIN, 9 * MID, _NW
    nk, nn = kdim // 128, ncols // _NT
    key = ("mrow", kdim, odim, ncols)
    if key not in _NC_CACHE:
        _NC_CACHE[key] = _build_mrow_nc(kdim, odim, ncols)
    nc = _NC_CACHE[key]
    nm = odim // 128
    lhsT = np.ascontiguousarray(
        wall.T.reshape(nk, 128, nm, 128).transpose(1, 2, 0, 3),
        dtype=bf16).reshape(128, nk * odim)
    xp = np.zeros((B, CIN, H + 2 * _HALO, W), np.float32)
    xp[:, :, _HALO:_HALO + H] = x
    in_maps = []
    for s in range(8):
        b, hh = s // 2, s % 2
        sh = xp[b, :, hh * 64: hh * 64 + _HS].reshape(CIN, ncols).astype(bf16)
        in_maps.append({
            "rhs": np.ascontiguousarray(
                sh.reshape(nk, 128, nn, _NT).transpose(2, 1, 0, 3)
            ).reshape(nn, 128, nk * _NT),
            "lhsT": lhsT,
        })
    t0 = time.perf_counter_ns()
    res = bass_utils.run_bass_kernel_spmd(nc, in_maps, core_ids=list(range(8)))
    t1 = time.perf_counter_ns()
    DEVICE_STATS.append({"wall_ns": t1 - t0,
                         "exec_time_ns": res.exec_time_ns})
    return [res.results[s]["out"].astype(np.float32) for s in range(8)]


def _taps_l1(x, wall):
    try:
        return _device_taps_l1(x, wall)
    except Exception as e:  # pragma: no cover - device fallback
        import traceback
        traceback.print_exc()
        print(f"[kernel] device path failed ({e!r}); numpy fallback")
        xp = np.zeros((B, CIN, H + 2 * _HALO, W), np.float32)
        xp[:, :, _HALO:_HALO + H] = x
        outs = []
        for s in range(8):
            b, hh = s // 2, s % 2
            sh = xp[b, :, hh * 64: hh * 64 + _HS].reshape(CIN, _NW)
            outs.append(wall @ sh)
        return outs


def _sample_u_l1(u_shards, om):
    """u_shards: 8 x [9*MID, _NW]; om [B,27,H,W] -> y1 [B, MID, HW]."""
    off_y = om[:, :K2]
    off_x = om[:, K2:2 * K2]
    mask = 1.0 / (1.0 + np.exp(-om[:, 2 * K2:]))
    y1 = np.empty((B, MID, H, W), np.float32)
    yy = np.arange(64, dtype=np.float32)[:, None]
    xx = np.arange(W, dtype=np.float32)[None, :]
    for s in range(8):
        b, hh = s // 2, s % 2
        u = u_shards[s].reshape(9, MID, _NW)
        ys = slice(hh * 64, hh * 64 + 64)
        acc = np.zeros((MID, 64, W), np.float32)
        for k in range(9):
            ply = yy + (_KY[k] + _HALO) + off_y[b, k, ys]  # local float row
            plx = xx + _KX[k] + off_x[b, k, ys]
            y0 = np.floor(ply)
            x0 = np.floor(plx)
            wy = ply - y0
            wx = plx - x0
            y0i = y0.astype(np.int32)
            x0i = x0.astype(np.int32)
            mk = mask[b, k, ys]
            uk = u[k]
            for dy in (0, 1):
                yi = np.clip(y0i + dy, 0, _HS - 1)
                wyv = wy if dy else 1.0 - wy
                for dx in (0, 1):
                    xi = x0i + dx
                    valid = (xi >= 0) & (xi < W)
                    idx = yi * W + np.clip(xi, 0, W - 1)
                    wxv = wx if dx else 1.0 - wx
                    coef = (mk * wyv * wxv * valid).astype(np.float32)
                    g = uk[:, idx.reshape(-1)].reshape(MID, 64, W)
                    acc += coef[None] * g
        y1[b, :, ys] = acc
    return y1.reshape(B, MID, H * W)


def _bn_relu(x, gamma, beta):
    """x [B,O,HW] -> same, sync-BN (biased var) + affine + relu."""
    mu = x.mean(axis=(0, 2), keepdims=True)
    var = ((x - mu) ** 2).mean(axis=(0, 2), keepdims=True)
    y = (x - mu) / np.sqrt(var + _EPS)
    y = y * gamma[None, :, None] + beta[None, :, None]
    return np.maximum(y, 0.0)


# ---------------------------------------------------------------- bass kernel
_NT = 512  # one PSUM bank of fp32 output columns


_NPS = 4  # obuf/store ring depth
_NPB = 8  # psum bank ring depth


def _ring_depth(nk, nn):
    """rhs slab ring depth: as many ~nk*1KB slabs as fit in ~150KB/partition.
    For L2 the whole rhs fits (ring never recycles -> no PE/DMA lockstep)."""
    return min(nn, (150 * 1024) // (nk * _NT * 2))


def _build_matmul_nc(kdim, odim, ncols):
    """out[odim, ncols] = lhsT.T @ rhs for pre-packed operands.

    bf16 operands / bf16 output, fp32 PSUM accumulation.

    Host pre-packs rhs as [nn, 128, nk*512] (slab-major) and lhsT as
    [128, nk*odim] so every DMA line is partition-contiguous (18KB rhs
    lines instead of 1KB strided lines -> descriptor-efficient DMA).
    sync streams rhs slabs (HWDGE FIFO => in-order completion => slab n
    ready once rd >= 16*(n+2)); PE runs nk-deep PSUM accumulation groups;
    DVE evicts PSUM -> SBUF bf16 and issues the out store on its own
    queue so stores never serialize with slab loads.
    """
    from contextlib import ExitStack

    import concourse.bass as bass
    import concourse.mybir as mybir

    f32 = mybir.dt.float32
    bf16 = mybir.dt.bfloat16
    nc = bass.Bass()
    nk, nm, nn = kdim // 128, odim // 128, ncols // _NT
    _NR = _ring_depth(nk, nn)
    rhs = nc.dram_tensor("rhs", [nn, 128, nk * _NT], bf16, kind="ExternalInput")
    lhsT = nc.dram_tensor("lhsT", [128, nk * odim], bf16, kind="ExternalInput")
    out = nc.dram_tensor("out", [odim, ncols], bf16, kind="ExternalOutput")

    with ExitStack() as es:
        wtile = es.enter_context(nc.sbuf_tensor("wtile", [128, nk * odim], bf16))
        rbufs = [es.enter_context(nc.sbuf_tensor(f"rbuf{i}", [128, nk * _NT], bf16))
                 for i in range(_NR)]
        obufs = [es.enter_context(nc.sbuf_tensor(f"obuf{i}", [128, _NT], bf16))
                 for i in range(_NPS)]
        psums = [es.enter_context(nc.psum_tensor(f"psum{i}", [128, _NT], f32))
                 for i in range(_NPB)]
        # A DMA's .then_inc(sem, 16) is 16 independent +1s (one per SDMA
        # engine slot), so a single shared counter cannot distinguish "slab
        # n fully done" from "mixed progress over slabs n-1..n+1" once
        # several DMAs are in flight. Dedicated sems per buffer slot make
        # "sem >= 16*pass" an exact completion test.
        wt = es.enter_context(nc.semaphore(name="wt"))
        wt2 = es.enter_context(nc.semaphore(name="wt2"))
        rdv = [es.enter_context(nc.semaphore(name=f"rdv{i}"))
               for i in range(_NR)]
        stv = [es.enter_context(nc.semaphore(name=f"stv{i}"))
               for i in range(_NPS)]
        pe = es.enter_context(nc.semaphore(name="pe"))
        dve = es.enter_context(nc.semaphore(name="dve"))
        # chunk slab 0 by k so the ramp's first accumulation group starts
        # on the first ~third of the slab instead of all of it; only safe
        # when rbufs never recycle (slab 0's sems aren't reused).
        chunk0 = _NR == nn and nk >= 3
        if chunk0:
            kch = [(j * nk // 3, (j + 1) * nk // 3) for j in range(3)]
            rdc = [es.enter_context(nc.semaphore(name=f"rdc{j}"))
                   for j in range(3)]
            kwait = {k0: j for j, (k0, _) in enumerate(kch)}
        block = es.enter_context(nc.Block(no_gpsimd_drain=True))

        ngroups = nn * nm

        @block.sync
        def _(sync):
            if chunk0:
                for j, (k0, k1) in enumerate(kch):
                    sync.dma_start(
                        rbufs[0][:, k0 * _NT:k1 * _NT],
                        rhs[0, :, k0 * _NT:k1 * _NT]).then_inc(rdc[j], 16)
            else:
                sync.dma_start(rbufs[0][:],
                               rhs[0, :, :]).then_inc(rdv[0], 16)
            for pre in range(1, min(_NR, nn)):
                sync.dma_start(rbufs[pre][:],
                               rhs[pre, :, :]).then_inc(rdv[pre], 16)
            for n in range(_NR, nn):
                # rbuf[n % _NR] is free once PE finished slab n - _NR
                sync.wait_ge(pe, (n - _NR + 1) * nm)
                sync.dma_start(rbufs[n % _NR][:],
                               rhs[n, :, :]).then_inc(rdv[n % _NR], 16)
            for i in range(_NPS):
                cnt = len(range(i, ngroups, _NPS))
                if cnt:
                    sync.wait_ge(stv[i], 16 * cnt)

        @block.tensor
        def _(tensor):
            tensor.wait_ge(wt, 16)
            for n in range(nn):
                if n == 0 and chunk0:
                    pass  # per-k-chunk waits inside the k loop below
                elif n == 0:
                    tensor.wait_ge(rdv[0], 16)
                else:
                    tensor.wait_ge(rdv[n % _NR], 16 * (n // _NR + 1))
                for m in range(nm):
                    g = n * nm + m
                    if n == 0 and m == 1:
                        tensor.wait_ge(wt2, 16)
                    if g >= _NPB:
                        tensor.wait_ge(dve, g - _NPB + 1)
                    ps = psums[g % _NPB]
                    mm = None
                    for k in range(nk):
                        if n == 0 and m == 0 and chunk0 and k in kwait:
                            tensor.wait_ge(rdc[kwait[k]], 16)
                        mm = tensor.matmul(
                            ps[:],
                            wtile[:, (m * nk + k) * 128:
                                  (m * nk + k + 1) * 128],
                            rbufs[n % _NR][:, k * _NT:(k + 1) * _NT],
                            start=(k == 0), stop=(k == nk - 1))
                    mm.then_inc(pe, 1)

        @block.vector
        def _(vector):
            for n in range(nn):
                for m in range(nm):
                    g = n * nm + m
                    vector.wait_ge(pe, g + 1)
                    if g >= _NPS:
                        vector.wait_ge(stv[g % _NPS], 16 * (g // _NPS))
                    vector.tensor_copy(
                        obufs[g % _NPS][:], psums[g % _NPB][:]).then_inc(dve, 1)

        @block.scalar
        def _(scalar):
            # out stores ride the Activation HWDGE ring so they never
            # serialize with the rhs slab loads on the SP ring; the wtile
            # load rides here too so it overlaps slab 0's load. lhsT is
            # packed m-major, so the m=0 weights land first (wt) and the
            # rest follows (wt2) -> the first group starts sooner.
            wc = nk * 128
            scalar.dma_start(wtile[:, :wc], lhsT[:, :wc]).then_inc(wt, 16)
            if nm > 1:
                scalar.dma_start(wtile[:, wc:], lhsT[:, wc:]).then_inc(wt2, 16)
            for n in range(nn):
                for m in range(nm):
                    g = n * nm + m
                    scalar.wait_ge(dve, g + 1)
                    scalar.dma_start(
                        out[m * 128:(m + 1) * 128, n * _NT:(n + 1) * _NT],
                        obufs[g % _NPS][:]).then_inc(stv[g % _NPS], 16)
    return nc


def _build_mrow_nc(kdim, odim, ncols):
    """out[odim, ncols] = lhsT.T @ rhs with the WHOLE rhs SBUF-resident.

    m-outer loop: for each 128-row output block m, sweep all nn column
    slabs (PSUM-accumulating over nk), evict into a [128, ncols] row
    buffer, and store the entire m-row as ONE DMA (ncols*2B contiguous
    per-partition lines). Used when rhs is small (L1 taps: rhs = x shard,
    nk*nn*1KB <= ~40KB/partition) but odim is large (9 taps * 128).

    With nk=2 the PSUM groups are tiny (~0.9us), so eviction throughput
    and blocking waits dominate if naive: use all 8 PSUM banks, split the
    evictions between the vector and scalar engines (even/odd group), and
    issue the 9 row stores from the sync ring (idle after the small x
    load). PE's bank-reuse waits are then almost always pre-satisfied and
    the matmuls stay back-to-back.
    """
    from contextlib import ExitStack

    import concourse.bass as bass
    import concourse.mybir as mybir

    f32 = mybir.dt.float32
    bf16 = mybir.dt.bfloat16
    nc = bass.Bass()
    nk, nm, nn = kdim // 128, odim // 128, ncols // _NT
    nps = 8
    rhs = nc.dram_tensor("rhs", [nn, 128, nk * _NT], bf16, kind="ExternalInput")
    lhsT = nc.dram_tensor("lhsT", [128, nk * odim], bf16, kind="ExternalInput")
    out = nc.dram_tensor("out", [odim, ncols], bf16, kind="ExternalOutput")

    with ExitStack() as es:
        wtile = es.enter_context(nc.sbuf_tensor("wtile", [128, nk * odim], bf16))
        rbufs = [es.enter_context(nc.sbuf_tensor(f"rbuf{i}", [128, nk * _NT], bf16))
                 for i in range(nn)]
        obig = [es.enter_context(nc.sbuf_tensor(f"obig{i}", [128, ncols], bf16))
                for i in range(2)]
        psums = [es.enter_context(nc.psum_tensor(f"psum{i}", [128, _NT], f32))
                 for i in range(nps)]
        wt = es.enter_context(nc.semaphore(name="wt"))
        wt2 = es.enter_context(nc.semaphore(name="wt2"))
        rdv = [es.enter_context(nc.semaphore(name=f"rdv{i}"))
               for i in range(nn)]
        stv = [es.enter_context(nc.semaphore(name=f"stv{i}"))
               for i in range(2)]
        pe = es.enter_context(nc.semaphore(name="pe"))
        dvev = es.enter_context(nc.semaphore(name="dvev"))
        dves = es.enter_context(nc.semaphore(name="dves"))
        block = es.enter_context(nc.Block(no_gpsimd_drain=True))

        def evict_done_wait(eng, g):
            """Wait until group g's eviction completed (whichever engine)."""
            if g % 2 == 0:
                eng.wait_ge(dvev, g // 2 + 1)
            else:
                eng.wait_ge(dves, g // 2 + 1)

        @block.sync
        def _(sync):
            for n in range(nn):
                sync.dma_start(rbufs[n][:], rhs[n, :, :]).then_inc(rdv[n], 16)
            for m in range(nm):
                ng = (m + 1) * nn
                sync.wait_ge(dvev, (ng + 1) // 2)
                sync.wait_ge(dves, ng // 2)
                sync.dma_start(
                    out[m * 128:(m + 1) * 128, :],
                    obig[m % 2][:]).then_inc(stv[m % 2], 16)

        @block.tensor
        def _(tensor):
            tensor.wait_ge(wt, 16)
            for m in range(nm):
                if m == 1:
                    tensor.wait_ge(wt2, 16)
                for n in range(nn):
                    g = m * nn + n
                    if m == 0:
                        tensor.wait_ge(rdv[n], 16)
                    if g >= nps:
                        evict_done_wait(tensor, g - nps)
                    ps = psums[g % nps]
                    mm = None
                    for k in range(nk):
                        mm = tensor.matmul(
                            ps[:],
                            wtile[:, (m * nk + k) * 128:
                                  (m * nk + k + 1) * 128],
                            rbufs[n][:, k * _NT:(k + 1) * _NT],
                            start=(k == 0), stop=(k == nk - 1))
                    mm.then_inc(pe, 1)

        @block.vector
        def _(vector):
            for m in range(nm):
                first = True
                for n in range(nn):
                    g = m * nn + n
                    if g % 2 != 0:
                        continue
                    if first and m >= 2:
                        vector.wait_ge(stv[m % 2], 16 * (m // 2))
                    first = False
                    vector.wait_ge(pe, g + 1)
                    vector.tensor_copy(
                        obig[m % 2][:, n * _NT:(n + 1) * _NT],
                        psums[g % nps][:]).then_inc(dvev, 1)

        @block.scalar
        def _(scalar):
            # lhsT packed m-major: m=0 weights first (wt), rest after (wt2)
            wc = nk * 128
            scalar.dma_start(wtile[:, :wc], lhsT[:, :wc]).then_inc(wt, 16)
            if nm > 1:
                scalar.dma_start(wtile[:, wc:], lhsT[:, wc:]).then_inc(wt2, 16)
            for m in range(nm):
                first = True
                for n in range(nn):
                    g = m * nn + n
                    if g % 2 != 1:
                        continue
                    if first and m >= 2:
                        scalar.wait_ge(stv[m % 2], 16 * (m // 2))
                    first = False
                    scalar.wait_ge(pe, g + 1)
                    scalar.copy(
                        obig[m % 2][:, n * _NT:(n + 1) * _NT],
                        psums[g % nps][:]).then_inc(dves, 1)
    return nc


_NC_CACHE = {}
DEVICE_STATS = []  # one entry per device invocation: {wall_ns, exec_time_ns}


def _device_contract(sampled, wr):
    """sampled [B, K, HW], wr [O, K] -> [B, O, HW] on 8 cores (b, hw-half)."""
    import time

    import ml_dtypes
    from concourse import bass_utils

    bf16 = np.dtype(ml_dtypes.bfloat16)
    bdim, kdim, hw = sampled.shape
    odim = wr.shape[0]
    half = hw // 2
    nk, nn = kdim // 128, half // _NT
    key = (kdim, odim, half)
    if key not in _NC_CACHE:
        _NC_CACHE[key] = _build_matmul_nc(kdim, odim, half)
    nc = _NC_CACHE[key]
    # lhsT packed [128, nk*odim]: partition p holds w[k*128+p, :] for all k
    nm = odim // 128
    lhsT = np.ascontiguousarray(
        wr.T.reshape(nk, 128, nm, 128).transpose(1, 2, 0, 3),
        dtype=bf16).reshape(128, nk * odim)
    # rhs packed [nn, 128, nk*_NT]: slab-major, partition-contiguous lines
    s16 = sampled.astype(bf16).reshape(bdim, nk, 128, 2, nn, _NT)
    in_maps = []
    for s in range(8):
        b, hh = s // 2, s % 2
        in_maps.append({
            "rhs": np.ascontiguousarray(
                s16[b, :, :, hh].transpose(2, 1, 0, 3)).reshape(
                    nn, 128, nk * _NT),
            "lhsT": lhsT,
        })
    t0 = time.perf_counter_ns()
    res = bass_utils.run_bass_kernel_spmd(nc, in_maps, core_ids=list(range(8)))
    t1 = time.perf_counter_ns()
    DEVICE_STATS.append({"wall_ns": t1 - t0,
                         "exec_time_ns": res.exec_time_ns})
    out = np.empty((bdim, odim, hw), dtype=np.float32)
    for s in range(8):
        b, hh = s // 2, s % 2
        out[b, :, hh * half:(hh + 1) * half] = \
            res.results[s]["out"].astype(np.float32)
    return out


def _contract(sampled, wr):
    try:
        return _device_contract(sampled, wr)
    except Exception as e:  # pragma: no cover - device fallback
        import traceback
        traceback.print_exc()
        print(f"[kernel] device path failed ({e!r}); numpy fallback")
        return np.matmul(wr[None], sampled)


# ---------------------------------------------------------------- entry point
def kernel(x, w_off1, b_off1, w1, b1, g1, be1,
           w_off2, b_off2, w2, b2, g2, be2):
    x = np.asarray(x, dtype=np.float32)

    om1 = _conv3x3_host(_im2col(x), np.asarray(w_off1),
                        np.asarray(b_off1)).reshape(B, 27, H, W)
    wall = np.ascontiguousarray(
        np.asarray(w1).reshape(MID, CIN, 9).transpose(2, 0, 1)
    ).reshape(9 * MID, CIN)
    u = _taps_l1(x, wall)
    y1 = _sample_u_l1(u, om1)
    y1 += np.asarray(b1)[None, :, None]
    h1 = _bn_relu(y1, np.asarray(g1), np.asarray(be1)).reshape(B, MID, H, W)

    s2 = _sampled_for_layer(h1, np.asarray(w_off2), np.asarray(b_off2))
    y2 = _contract(s2, np.asarray(w2).reshape(COUT, -1))
    y2 += np.asarray(b2)[None, :, None]
    h2 = _bn_relu(y2, np.asarray(g2), np.asarray(be2)).reshape(B, COUT, H, W)
    return h2

